# revision 11
# baseline (speedup 1.0000x reference)
"""AttentionPooling (segment softmax pooling) Trainium2 kernel.

Math (per reference):
    h = tanh(x @ W1 + b1); s = h @ W2 + b2
    w = softmax(s) within each contiguous segment (batch is sorted)
    out[b] = sum_{r in b} w_r * x[r]

Device algorithm (per core, segments sharded 512/core):
  Softmax is shift-invariant and |s| <= ||W2||_1 + |b2| ~ 9, so we skip the
  per-segment max and use e_r = exp(s_r + b2) directly (safe in fp32).
  out[b] = (sum e_r x_r) / (sum e_r): both sums come from one-hot matmuls
  contracted over rows, accumulated in PSUM over a W_SEG-segment group
  window, then scatter-accumulated (indirect DMA, compute_op=add) into a
  DRAM scratch [segs, 257] (256 pooled cols + 1 sum col); a final pass
  divides.

  Scores need x^T (D on partitions): host supplies x^T in bf16 (score path
  only shapes softmax weights; bf16 there perturbs the output by ~1e-3
  relative). Pooling reads x in natural layout (bf16).

  The esel one-hot selector for a whole 2048-row group is built in two
  group-wide elementwise ops using stride-0 broadcast access patterns
  (is_equal on gpsimd, multiply-by-e on DVE) rather than per-tile ops —
  per-instruction overhead on DVE/GpSimd dominated the previous version.

The program is identical across cores (SPMD); all data-dependent segment
offsets travel through input tensors (batch_local window ids + scatter row
indices), never through baked constants.
"""

import os
from contextlib import ExitStack

import numpy as np
import ml_dtypes

LAST_EXEC_NS = None

import concourse.bass as bass
import concourse.bacc as bacc
import concourse.tile as tile
from concourse import mybir
from concourse.bass import IndirectOffsetOnAxis, broadcast_tensor_aps
from concourse.bass_utils import run_bass_kernel_spmd

# ---- problem constants (hardcoded per contract) ----
N_TOTAL = 500000
D = 256
H = 128
NUM_SEGMENTS = 4096
N_CORES = 8
SEGS_PER_CORE = NUM_SEGMENTS // N_CORES  # 512

G_ROWS = 2048          # rows per group
TILES_PER_G = 16       # 128-row tiles per group
SUB_PER_G = 4          # 512-row subtiles per group (score matmuls)
SCRATCH_ROWS = 640     # 512 real segs + 128 pad rows for window overflow
PAD_BL = 255.0         # batch_local value for padding rows (never matches iota)

F32 = mybir.dt.float32
BF16 = mybir.dt.bfloat16
I32 = mybir.dt.int32


def build_nc(
    n_groups: int, b2_val: float, w_seg: int, fin_after: list[int]
) -> bass.Bass:
    """fin_after[b] = group index after whose scatter the final-normalize
    chunk b (segment rows 128b..128b+127) may be read (max over cores)."""
    r_pad = n_groups * G_ROWS
    n_tiles = n_groups * TILES_PER_G

    nc = bacc.Bacc("TRN2", target_bir_lowering=False, debug=False)

    # DRAM I/O
    # x_nat carries D cols of x, a ones column (col 256, folds the seg_sum
    # matmul into the pooling matmul), and a zero pad col. Layout is
    # partition-major [128, n_tiles, 258]: x_nat[p, t, :] = x[128t + p, :],
    # so one group's load is a single contiguous 8.2KB run per partition.
    x_nat = nc.dram_tensor("x_nat", [128, n_tiles, D + 2], BF16, kind="ExternalInput")
    xT = nc.dram_tensor("xT", [D, r_pad], BF16, kind="ExternalInput")
    w1c = nc.dram_tensor("w1c", [2, 128, H], BF16, kind="ExternalInput")
    w2col = nc.dram_tensor("w2col", [H, 1], BF16, kind="ExternalInput")
    b1col = nc.dram_tensor("b1col", [H, 1], F32, kind="ExternalInput")
    iotaw = nc.dram_tensor("iotaw", [128, w_seg], BF16, kind="ExternalInput")
    bl_all = nc.dram_tensor("bl_all", [128, n_tiles], F32, kind="ExternalInput")
    seg_idx = nc.dram_tensor("seg_idx", [w_seg, n_groups], I32, kind="ExternalInput")
    # ExternalOutput buffers are zero-initialized by the runtime — scratch
    # relies on that for its scatter-accumulate
    scratch = nc.dram_tensor("scratch", [SCRATCH_ROWS, 257], F32, kind="ExternalOutput")
    out = nc.dram_tensor("out", [SCRATCH_ROWS, D], F32, kind="ExternalOutput")

    with tile.TileContext(nc) as tc, ExitStack() as ctx:
        const_pool = ctx.enter_context(tc.tile_pool(name="const", bufs=1))
        xT_pool = ctx.enter_context(tc.tile_pool(name="xT", bufs=6))
        xnat_pool = ctx.enter_context(tc.tile_pool(name="xnat", bufs=6))
        h_pool = ctx.enter_context(tc.tile_pool(name="h", bufs=2))
        e_pool = ctx.enter_context(tc.tile_pool(name="e", bufs=3))
        mask_pool = ctx.enter_context(tc.tile_pool(name="mask", bufs=3))
        esel_pool = ctx.enter_context(tc.tile_pool(name="esel", bufs=3))
        flush_pool = ctx.enter_context(tc.tile_pool(name="flush", bufs=2))
        fin_pool = ctx.enter_context(tc.tile_pool(name="fin", bufs=2))
        u_psum = ctx.enter_context(tc.tile_pool(name="u_ps", bufs=1, space="PSUM"))
        s_psum = ctx.enter_context(tc.tile_pool(name="s_ps", bufs=2, space="PSUM"))
        p_psum = ctx.enter_context(tc.tile_pool(name="p_ps", bufs=2, space="PSUM"))

        # ---- constants ----
        w1c_t = const_pool.tile([128, 2 * H], BF16, tag="w1c")
        nc.sync.dma_start(w1c_t[:, 0:H], w1c[0])
        nc.sync.dma_start(w1c_t[:, H : 2 * H], w1c[1])
        w2_t = const_pool.tile([H, 1], BF16, tag="w2")
        nc.sync.dma_start(w2_t[:], w2col[:, :])
        b1_t = const_pool.tile([H, 1], F32, tag="b1")
        nc.sync.dma_start(b1_t[:], b1col[:, :])
        iota_t = const_pool.tile([128, w_seg], BF16, tag="iota")
        nc.sync.dma_start(iota_t[:], iotaw[:, :])
        bl_t = const_pool.tile([128, n_tiles], F32, tag="bl")
        nc.sync.dma_start(bl_t[:], bl_all[:, :])
        sidx_t = const_pool.tile([w_seg, n_groups], I32, tag="sidx")
        nc.sync.dma_start(sidx_t[:], seg_idx[:, :])

        # per-group pipeline state carried across loop iterations
        prev = None  # (esel_tile, xn_tile, g)

        def normalize_chunk(b):
            """out[128b:128b+128] = scratch[...,:256] / scratch[...,256]."""
            ft = fin_pool.tile([128, 257], F32, tag="ft")
            nc.gpsimd.dma_start(ft[:], scratch[128 * b : 128 * (b + 1), :])
            rec = fin_pool.tile([128, 1], F32, tag="rec")
            eps = fin_pool.tile([128, 1], F32, tag="eps")
            nc.vector.tensor_scalar(
                eps[:], ft[:, D : D + 1], 1e-30, None, mybir.AluOpType.add,
            )
            nc.vector.reciprocal(rec[:], eps[:])
            ot = fin_pool.tile([128, D], F32, tag="ot")
            nc.vector.tensor_scalar(
                ot[:], ft[:, 0:D], rec[:, 0:1], None, mybir.AluOpType.mult,
            )
            nc.sync.dma_start(out[128 * b : 128 * (b + 1), :], ot[:])

        def flush_window(esel, xn, g):
            """Pooling matmuls for group g (2-way col-tiled pairs run on
            concurrent PE subarrays), then scatter-accumulate."""
            pooled = p_psum.tile([2 * w_seg, 257], F32, tag="pooled")
            for c in range(TILES_PER_G):
                half = c % 2
                nc.tensor.matmul(
                    pooled[half * w_seg : (half + 1) * w_seg, 0:257],
                    esel[:, c * w_seg : (c + 1) * w_seg],
                    xn[:, c * (D + 2) : c * (D + 2) + 257],
                    start=(c < 2),
                    stop=(c >= TILES_PER_G - 2),
                    skip_group_check=True,
                    tile_position=(0, half * w_seg),
                )
            fl = flush_pool.tile([w_seg, 257], F32, tag="fl")
            nc.vector.tensor_copy(fl[:], pooled[0:w_seg, :])
            nc.vector.tensor_tensor(
                fl[:], fl[:], pooled[w_seg : 2 * w_seg, :], mybir.AluOpType.add
            )
            nc.gpsimd.indirect_dma_start(
                scratch[:, :],
                IndirectOffsetOnAxis(ap=sidx_t[:, g : g + 1], axis=0),
                fl[:],
                None,
                compute_op=mybir.AluOpType.add,
            )
            for b in range(5):
                if fin_after[b] == g:
                    normalize_chunk(b)

        # ---- main loop over row groups ----
        for g in range(n_groups):
            # both 128-partition halves of x^T for this group in one DMA
            xt = xT_pool.tile([128, 2 * G_ROWS], BF16, tag="xt")
            nc.sync.dma_start(
                xt[:].rearrange("p (h r) -> p h r", h=2),
                xT[:, g * G_ROWS : (g + 1) * G_ROWS].rearrange(
                    "(h p) r -> p h r", h=2
                ),
            )
            xn = xnat_pool.tile([128, TILES_PER_G * (D + 2)], BF16, tag="xn")
            t0 = g * TILES_PER_G
            nc.scalar.dma_start(
                xn[:].rearrange("p (t d) -> p t d", d=D + 2),
                x_nat[:, t0 : t0 + TILES_PER_G, :],
            )

            # scores: u_i = W1a^T xt0_i + W1b^T xt1_i per 512-row subtile,
            # ordered all-W1a then all-W1b so the stationary weight only
            # changes twice per group.
            u_tiles = [
                u_psum.tile([H, 512], F32, tag=f"u{i}", name=f"u{i}")
                for i in range(SUB_PER_G)
            ]
            for i in range(SUB_PER_G):
                sl = slice(512 * i, 512 * (i + 1))
                nc.tensor.matmul(
                    u_tiles[i][:], w1c_t[:, 0:H], xt[:, sl],
                    start=True, stop=False, skip_group_check=True,
                )
            h_t = h_pool.tile([H, G_ROWS], BF16, tag="h")
            for i in range(SUB_PER_G):
                sl = slice(512 * i, 512 * (i + 1))
                nc.tensor.matmul(
                    u_tiles[i][:], w1c_t[:, H : 2 * H],
                    xt[:, G_ROWS + 512 * i : G_ROWS + 512 * (i + 1)],
                    start=False, stop=True, skip_group_check=True,
                )
                nc.scalar.activation(
                    h_t[:, sl], u_tiles[i][:],
                    mybir.ActivationFunctionType.Tanh, bias=b1_t[:, 0:1],
                )

            # interleave previous group's pooling here: it fills the PE
            # while tanh/exp/esel for this group run on Act/DVE/GpSimd.
            if prev is not None:
                flush_window(*prev)

            # snat[p, c] = score of row 128c + p (pre-bias)
            snat = s_psum.tile([128, TILES_PER_G], F32, tag="snat")
            for c in range(TILES_PER_G):
                nc.tensor.matmul(
                    snat[:, c : c + 1],
                    h_t[:, 128 * c : 128 * (c + 1)],
                    w2_t[:],
                    start=(c == 0),
                    stop=(c == TILES_PER_G - 1),
                    skip_group_check=True,
                )
            e_t = e_pool.tile([128, TILES_PER_G], F32, tag="e")
            nc.scalar.activation(
                e_t[:], snat[:], mybir.ActivationFunctionType.Exp, bias=float(b2_val)
            )

            # group-wide esel: mask[p,c,s] = (iota[s] == bl[p, t0+c]);
            # esel[p,c,s] = mask * e[p,c]. Broadcast via stride-0 APs.
            mask = mask_pool.tile([128, TILES_PER_G * w_seg], BF16, tag="mask")
            mask3 = mask[:].rearrange("p (c s) -> p c s", s=w_seg)
            iota3 = iota_t[:].rearrange("p (o s) -> p o s", o=1)
            bl3 = bl_t[:, t0 : t0 + TILES_PER_G].rearrange("p (c o) -> p c o", o=1)
            i_b, b_b = broadcast_tensor_aps(iota3, bl3)
            nc.vector.tensor_tensor(mask3, i_b, b_b, mybir.AluOpType.is_equal)
            esel = esel_pool.tile([128, TILES_PER_G * w_seg], BF16, tag="esel")
            esel3 = esel[:].rearrange("p (c s) -> p c s", s=w_seg)
            e3 = e_t[:].rearrange("p (c o) -> p c o", o=1)
            m_b, e_b = broadcast_tensor_aps(mask3, e3)
            nc.vector.tensor_tensor(esel3, m_b, e_b, mybir.AluOpType.mult)

            prev = (esel, xn, g)

        flush_window(*prev)

    return nc


def kernel(x, batch, W1, b1, W2, b2):
    x = np.asarray(x, dtype=np.float32)
    batch = np.asarray(batch)
    W1 = np.asarray(W1, dtype=np.float32)
    b1 = np.asarray(b1, dtype=np.float32)
    W2 = np.asarray(W2, dtype=np.float32)
    b2 = np.asarray(b2, dtype=np.float32)
    n, d = x.shape
    assert d == D

    bounds = np.searchsorted(batch, np.arange(NUM_SEGMENTS + 1))
    core_starts = [int(bounds[SEGS_PER_CORE * m]) for m in range(N_CORES + 1)]
    rows_per_core = [core_starts[m + 1] - core_starts[m] for m in range(N_CORES)]
    n_groups = max(1, int(np.ceil(max(rows_per_core) / G_ROWS)))
    r_pad = n_groups * G_ROWS
    n_tiles = n_groups * TILES_PER_G

    # window width: 32 segs if every group's span fits unaligned, else 64
    # (32-aligned start).
    max_span = 0
    for m in range(N_CORES):
        rs, re = core_starts[m], core_starts[m + 1]
        seg_local = batch[rs:re] - SEGS_PER_CORE * m
        rows = re - rs
        for g in range(n_groups):
            lo = g * G_ROWS
            hi = min((g + 1) * G_ROWS, rows)
            if lo < rows:
                max_span = max(
                    max_span, int(seg_local[hi - 1]) - int(seg_local[lo])
                )
    w_seg = 32 if max_span < 32 else 64

    # fin_after[b]: last group (max over cores) whose scatter window can
    # touch segment rows [128b, 128b+128) — the normalize for that chunk is
    # issued right after that group's scatter.
    fin_after = [0] * 5
    for m in range(N_CORES):
        rs, re = core_starts[m], core_starts[m + 1]
        seg_local = batch[rs:re] - SEGS_PER_CORE * m
        rows = re - rs
        for g in range(n_groups):
            lo = g * G_ROWS
            if lo >= rows:
                s0 = SEGS_PER_CORE
            elif w_seg == 32:
                s0 = int(seg_local[lo])
            else:
                s0 = 32 * (int(seg_local[lo]) // 32)
            for b in range(5):
                if s0 < 128 * (b + 1) and s0 + w_seg > 128 * b:
                    fin_after[b] = max(fin_after[b], g)

    # shared constant inputs
    w1c = np.ascontiguousarray(W1.reshape(2, 128, H).astype(ml_dtypes.bfloat16))
    w2col = np.ascontiguousarray(W2.reshape(H, 1).astype(ml_dtypes.bfloat16))
    b1col = np.ascontiguousarray(b1.reshape(H, 1))
    iotaw = np.broadcast_to(np.arange(w_seg), (128, w_seg)).astype(ml_dtypes.bfloat16)
    b2_val = float(b2.reshape(-1)[0])

    in_maps = []
    for m in range(N_CORES):
        rs, re = core_starts[m], core_starts[m + 1]
        rows = re - rs
        xm = x[rs:re]
        x_flat = np.zeros((r_pad, D + 2), dtype=ml_dtypes.bfloat16)
        x_flat[:rows, :D] = xm.astype(ml_dtypes.bfloat16)
        x_flat[:rows, D] = ml_dtypes.bfloat16(1.0)
        # partition-major: x_nat[p, t, :] = x_flat[128t + p, :]
        x_nat = np.ascontiguousarray(
            x_flat.reshape(n_tiles, 128, D + 2).transpose(1, 0, 2)
        )
        xT = np.zeros((D, r_pad), dtype=ml_dtypes.bfloat16)
        xT[:, :rows] = xm.T.astype(ml_dtypes.bfloat16)

        seg_local = (batch[rs:re] - SEGS_PER_CORE * m).astype(np.int64)
        assert seg_local.min() >= 0 and seg_local.max() < SEGS_PER_CORE

        bl = np.full((128, n_tiles), PAD_BL, dtype=np.float32)
        sidx = np.empty((w_seg, n_groups), dtype=np.int32)
        for g in range(n_groups):
            lo = g * G_ROWS
            hi = min((g + 1) * G_ROWS, rows)
            if lo >= rows:
                s0 = SEGS_PER_CORE  # pad region
            else:
                if w_seg == 32:
                    s0 = int(seg_local[lo])
                else:
                    s0 = 32 * (int(seg_local[lo]) // 32)
                span = int(seg_local[hi - 1]) - s0
                assert span < w_seg, f"group seg span {span} >= {w_seg}"
                rr = np.arange(lo, hi)
                p = rr % 128
                c = (rr % G_ROWS) // 128
                bl[p, g * TILES_PER_G + c] = (seg_local[lo:hi] - s0).astype(np.float32)
            sidx[:, g] = s0 + np.arange(w_seg)
        in_maps.append(
            {
                "x_nat": x_nat,
                "xT": xT,
                "w1c": w1c,
                "w2col": w2col,
                "b1col": b1col,
                "iotaw": iotaw,
                "bl_all": bl,
                "seg_idx": sidx,
            }
        )

    nc = build_nc(n_groups, b2_val, w_seg, fin_after)
    if not nc.is_finalized():
        nc.finalize()
    trace = os.environ.get("KERNEL_TRACE", "0") == "1"
    kw = {}
    if trace:
        kw = dict(trace=True, tmpdir=os.environ.get("KERNEL_TRACE_DIR") or None)
    res = run_bass_kernel_spmd(nc, in_maps, core_ids=list(range(N_CORES)), **kw)
    global LAST_EXEC_NS
    LAST_EXEC_NS = res.exec_time_ns
    if trace:
        print(
            f"exec_time_ns={res.exec_time_ns} mean={res.mean_exec_time_ns} "
            f"max_core={res.max_exec_time_core_id}",
            flush=True,
        )
    outs = res.results

    full = np.empty((NUM_SEGMENTS, D), dtype=np.float32)
    for m in range(N_CORES):
        full[SEGS_PER_CORE * m : SEGS_PER_CORE * (m + 1)] = outs[m]["out"][
            :SEGS_PER_CORE
        ]
    return full


# revision 44
# speedup vs baseline: 1.3144x; 1.3144x over previous
"""AttentionPooling (segment softmax pooling) Trainium2 kernel.

Math (per reference):
    h = tanh(x @ W1 + b1); s = h @ W2 + b2
    w = softmax(s) within each contiguous segment (batch is sorted)
    out[b] = sum_{r in b} w_r * x[r]

Device algorithm (per core, segments sharded 512/core):
  Softmax is shift-invariant and |s| <= ||W2||_1 + |b2| ~ 9, so we skip the
  per-segment max and use e_r = exp(s_r + b2) directly (safe in fp32).
  out[b] = (sum e_r x_r) / (sum e_r): both sums come from one-hot matmuls
  contracted over rows (a ones column folds the seg_sum matmul into the
  pooling matmul), accumulated in PSUM over each 2048-row group's W_SEG-
  segment window, then parked in persistent SBUF accumulator tiles (4
  group-windows per 128-partition tile). Window overlaps between groups are
  resolved at the end by a second one-hot matmul pass (absolute segment id
  of each accumulator row vs segment iota), followed by the normalization
  divide — no DRAM scratch, no indirect-DMA scatter.

  Scores need x^T (D on partitions): host supplies x^T in bf16 (score path
  only shapes softmax weights; bf16 there perturbs the output by ~1e-3
  relative). Pooling reads x in natural layout (bf16).

  The esel one-hot selector for a whole group is built in two group-wide
  elementwise DVE ops using stride-0 broadcast access patterns; pooling
  matmuls run as 2-way col-tiled pairs on concurrent PE subarrays, and
  consume esel with a 2-group pipeline lag so the exp -> mask -> esel chain
  never stalls the PE.

The program is identical across cores (SPMD); all data-dependent segment
offsets travel through input tensors (batch_local window ids + accumulator
row segment ids), never through baked constants.
"""

import os
from contextlib import ExitStack

import numpy as np
import ml_dtypes

LAST_EXEC_NS = None

import concourse.bass as bass
import concourse.bacc as bacc
import concourse.tile as tile
from concourse import mybir
from concourse.bass import broadcast_tensor_aps
from concourse.bass_utils import run_bass_kernel_spmd

# ---- problem constants (hardcoded per contract) ----
N_TOTAL = 500000
D = 256
H = 128
NUM_SEGMENTS = 4096
N_CORES = 8
SEGS_PER_CORE = NUM_SEGMENTS // N_CORES  # 512

G_ROWS = 2048          # rows per group
TILES_PER_G = 16       # 128-row tiles per group
SUB_PER_G = 4          # 512-row subtiles per group (score matmuls)
PAD_BL = 255.0         # batch_local value for padding rows (never matches iota)
PAD_SEG = 100000.0     # acc-row segment id for pad windows (never selected)

F32 = mybir.dt.float32
F32R = mybir.dt.float32r
BF16 = mybir.dt.bfloat16


def build_nc(n_groups: int, b2_val: float, w_seg: int) -> bass.Bass:
    r_pad = n_groups * G_ROWS
    n_tiles = n_groups * TILES_PER_G
    wins_per_acc = 128 // w_seg               # group windows per acc tile
    n_acc = -(-n_groups // wins_per_acc)      # acc tiles
    n_segb = SEGS_PER_CORE // 128             # 128-seg output chunks

    nc = bacc.Bacc("TRN2", target_bir_lowering=False, debug=False)

    # DRAM I/O
    # x_nat carries D cols of x, a ones column (col 256, folds the seg_sum
    # matmul into the pooling matmul), and a zero pad col. Layout is
    # partition-major [128, n_tiles, 258]: x_nat[p, t, :] = x[128t + p, :],
    # so one group's load is a single contiguous 8.2KB run per partition.
    x_nat = nc.dram_tensor("x_nat", [128, n_tiles, D + 2], BF16, kind="ExternalInput")
    xT = nc.dram_tensor("xT", [D, r_pad], BF16, kind="ExternalInput")
    w1c = nc.dram_tensor("w1c", [2, 128, H], BF16, kind="ExternalInput")
    w2col = nc.dram_tensor("w2col", [H, 1], BF16, kind="ExternalInput")
    b1col = nc.dram_tensor("b1col", [H, 1], F32, kind="ExternalInput")
    iotaw = nc.dram_tensor("iotaw", [128, w_seg], BF16, kind="ExternalInput")
    bl_all = nc.dram_tensor("bl_all", [128, n_tiles], F32, kind="ExternalInput")
    # crow[p, t] = absolute segment id of accumulator row (t, p)
    crow = nc.dram_tensor("crow", [128, n_acc], F32, kind="ExternalInput")
    # iotas[p, b*128 + j] = 128b + j (same for all partitions)
    iotas = nc.dram_tensor("iotas", [128, n_segb * 128], F32, kind="ExternalInput")
    zpad = nc.dram_tensor("zpad", [128, 257], F32, kind="ExternalInput")
    out = nc.dram_tensor("out", [SEGS_PER_CORE, D], F32, kind="ExternalOutput")

    with tile.TileContext(nc) as tc, ExitStack() as ctx:
        const_pool = ctx.enter_context(tc.tile_pool(name="const", bufs=1))
        acc_pool = ctx.enter_context(tc.tile_pool(name="acc", bufs=1))
        xT_pool = ctx.enter_context(tc.tile_pool(name="xT", bufs=8))
        xnat_pool = ctx.enter_context(tc.tile_pool(name="xnat", bufs=8))
        h_pool = ctx.enter_context(tc.tile_pool(name="h", bufs=2))
        e_pool = ctx.enter_context(tc.tile_pool(name="e", bufs=3))
        mask_pool = ctx.enter_context(tc.tile_pool(name="mask", bufs=3))
        esel_pool = ctx.enter_context(tc.tile_pool(name="esel", bufs=5))
        fin_pool = ctx.enter_context(tc.tile_pool(name="fin", bufs=2))
        u_psum = ctx.enter_context(tc.tile_pool(name="u_ps", bufs=1, space="PSUM"))
        s_psum = ctx.enter_context(tc.tile_pool(name="s_ps", bufs=2, space="PSUM"))
        p_psum = ctx.enter_context(tc.tile_pool(name="p_ps", bufs=2, space="PSUM"))
        o_psum = ctx.enter_context(tc.tile_pool(name="o_ps", bufs=1, space="PSUM"))

        # ---- constants ----
        w1c_t = const_pool.tile([128, 2 * H], BF16, tag="w1c")
        nc.sync.dma_start(w1c_t[:, 0:H], w1c[0])
        nc.sync.dma_start(w1c_t[:, H : 2 * H], w1c[1])
        w2_t = const_pool.tile([H, 1], BF16, tag="w2")
        nc.sync.dma_start(w2_t[:], w2col[:, :])
        b1_t = const_pool.tile([H, 1], F32, tag="b1")
        nc.sync.dma_start(b1_t[:], b1col[:, :])
        iota_t = const_pool.tile([128, w_seg], BF16, tag="iota")
        nc.sync.dma_start(iota_t[:], iotaw[:, :])
        bl_t = const_pool.tile([128, n_tiles], F32, tag="bl")
        nc.sync.dma_start(bl_t[:], bl_all[:, :])
        crow_t = const_pool.tile([128, n_acc], F32, tag="crow")
        nc.sync.dma_start(crow_t[:], crow[:, :])
        iotas_t = const_pool.tile([128, n_segb * 128], F32, tag="iotas")
        nc.sync.dma_start(iotas_t[:], iotas[:, :])

        # persistent per-core window accumulators: acc[t][w_seg*j + k, :] =
        # pooled window row k of group (wins_per_acc*t + j)
        acc_tiles = [
            acc_pool.tile([128, 257], F32, tag=f"acc{t}", name=f"acc{t}")
            for t in range(n_acc)
        ]
        # zero-init rows not covered by any group's flush: they are read
        # (x0) by the combine matmuls, and stale SBUF could hold NaN/Inf
        rem = n_groups - (n_acc - 1) * wins_per_acc
        if rem * w_seg < 128:
            nc.sync.dma_start(
                acc_tiles[n_acc - 1][rem * w_seg : 128, :],
                zpad[rem * w_seg : 128, :],
            )

        # pooling lags the score pipeline by PIPE_LAG groups, and the
        # mask/esel build for a group runs one iteration after its exp —
        # so the Vector queue never waits and never stalls the PE.
        PIPE_LAG = 3
        pending_e = []     # [(e_tile, t0, xn_tile, g), ...] awaiting esel build
        pending_fl = []    # [(esel_tile, xn_tile, g), ...] awaiting pooling

        def build_esel(e_t, t0, xn, g):
            """mask[p,c,s] = (iota[s] == bl[p, t0+c]);
            esel[p,c,s] = mask * e[p,c]. Broadcast via stride-0 APs."""
            mask = mask_pool.tile([128, TILES_PER_G * w_seg], BF16, tag="mask")
            mask3 = mask[:].rearrange("p (c s) -> p c s", s=w_seg)
            iota3 = iota_t[:].rearrange("p (o s) -> p o s", o=1)
            bl3 = bl_t[:, t0 : t0 + TILES_PER_G].rearrange("p (c o) -> p c o", o=1)
            i_b, b_b = broadcast_tensor_aps(iota3, bl3)
            nc.vector.tensor_tensor(mask3, i_b, b_b, mybir.AluOpType.is_equal)
            esel = esel_pool.tile([128, TILES_PER_G * w_seg], BF16, tag="esel")
            esel3 = esel[:].rearrange("p (c s) -> p c s", s=w_seg)
            e3 = e_t[:].rearrange("p (c o) -> p c o", o=1)
            m_b, e_b = broadcast_tensor_aps(mask3, e3)
            nc.vector.tensor_tensor(esel3, m_b, e_b, mybir.AluOpType.mult)
            pending_fl.append((esel, xn, g))

        def flush_window(esel, xn, g):
            """Pooling matmuls for group g (2-way col-tiled pairs on
            concurrent PE subarrays), then park the window in SBUF."""
            pooled = p_psum.tile([2 * w_seg, 257], F32, tag="pooled")
            for c in range(TILES_PER_G):
                half = c % 2
                nc.tensor.matmul(
                    pooled[half * w_seg : (half + 1) * w_seg, 0:257],
                    esel[:, c * w_seg : (c + 1) * w_seg],
                    xn[:, c * (D + 2) : c * (D + 2) + 257],
                    start=(c < 2),
                    stop=(c >= TILES_PER_G - 2),
                    skip_group_check=True,
                    tile_position=(0, half * w_seg),
                )
            at = acc_tiles[g // wins_per_acc]
            j = g % wins_per_acc
            sl = slice(w_seg * j, w_seg * (j + 1))
            nc.vector.tensor_copy(at[sl, :], pooled[0:w_seg, :])
            nc.vector.tensor_tensor(
                at[sl, :], at[sl, :], pooled[w_seg : 2 * w_seg, :],
                mybir.AluOpType.add,
            )

        # ---- main loop over row groups ----
        for g in range(n_groups):
            # both 128-partition halves of x^T for this group in one DMA
            xt = xT_pool.tile([128, 2 * G_ROWS], BF16, tag="xt")
            nc.sync.dma_start(
                xt[:].rearrange("p (h r) -> p h r", h=2),
                xT[:, g * G_ROWS : (g + 1) * G_ROWS].rearrange(
                    "(h p) r -> p h r", h=2
                ),
            )
            xn = xnat_pool.tile([128, TILES_PER_G * (D + 2)], BF16, tag="xn")
            t0 = g * TILES_PER_G
            nc.scalar.dma_start(
                xn[:].rearrange("p (t d) -> p t d", d=D + 2),
                x_nat[:, t0 : t0 + TILES_PER_G, :],
            )

            # build esel for the group whose exp completed last iteration:
            # its inputs are ready, so the Vector queue starts immediately
            if pending_e:
                build_esel(*pending_e.pop(0))

            # scores: u_i = W1a^T xt0_i + W1b^T xt1_i per 512-row subtile,
            # ordered all-W1a then all-W1b so the stationary weight only
            # changes twice per group.
            u_tiles = [
                u_psum.tile([H, 512], F32, tag=f"u{i % 3}", name=f"u{i}")
                for i in range(SUB_PER_G)
            ]
            for i in range(SUB_PER_G):
                sl = slice(512 * i, 512 * (i + 1))
                nc.tensor.matmul(
                    u_tiles[i][:], w1c_t[:, 0:H], xt[:, sl],
                    start=True, stop=False, skip_group_check=True,
                )
            h_t = h_pool.tile([H, G_ROWS], BF16, tag="h")
            for i in range(SUB_PER_G):
                sl = slice(512 * i, 512 * (i + 1))
                nc.tensor.matmul(
                    u_tiles[i][:], w1c_t[:, H : 2 * H],
                    xt[:, G_ROWS + 512 * i : G_ROWS + 512 * (i + 1)],
                    start=False, stop=True, skip_group_check=True,
                )
                nc.scalar.activation(
                    h_t[:, sl], u_tiles[i][:],
                    mybir.ActivationFunctionType.Tanh, bias=b1_t[:, 0:1],
                )

            # interleave a lagged group's pooling here: it fills the PE
            # while tanh/exp/esel for this group run on Act/DVE.
            if len(pending_fl) >= PIPE_LAG:
                flush_window(*pending_fl.pop(0))

            # snat[p, c] = score of row 128c + p (pre-bias)
            snat = s_psum.tile([128, TILES_PER_G], F32, tag="snat")
            for c in range(TILES_PER_G):
                nc.tensor.matmul(
                    snat[:, c : c + 1],
                    h_t[:, 128 * c : 128 * (c + 1)],
                    w2_t[:],
                    start=(c == 0),
                    stop=(c == TILES_PER_G - 1),
                    skip_group_check=True,
                )
            e_t = e_pool.tile([128, TILES_PER_G], F32, tag="e")
            nc.scalar.activation(
                e_t[:], snat[:], mybir.ActivationFunctionType.Exp, bias=float(b2_val)
            )
            pending_e.append((e_t, t0, xn, g))

        while pending_e:
            build_esel(*pending_e.pop(0))
        for item in pending_fl:
            flush_window(*item)

        # ---- combine windows + normalize ----
        # split each f32 accumulator into bf16 hi + lo (exact to ~16
        # mantissa bits) so the combine runs as bf16 matmuls
        ahi_tiles, alo_tiles = [], []
        for t in range(n_acc):
            ahi = acc_pool.tile([128, 257], BF16, tag=f"ahi{t}", name=f"ahi{t}")
            nc.vector.tensor_copy(ahi[:], acc_tiles[t][:])
            alo = acc_pool.tile([128, 257], BF16, tag=f"alo{t}", name=f"alo{t}")
            nc.vector.tensor_tensor(
                alo[:], acc_tiles[t][:], ahi[:], mybir.AluOpType.subtract
            )
            ahi_tiles.append(ahi)
            alo_tiles.append(alo)
        # C_b[p, t, j] = (crow[p, t] == 128b + j): one-hot combine masks
        crow3 = crow_t[:].rearrange("p (t o) -> p t o", o=1)
        for b in range(n_segb):
            cb = fin_pool.tile([128, n_acc * 128], BF16, tag="cb")
            cb3 = cb[:].rearrange("p (t j) -> p t j", j=128)
            io3 = iotas_t[:, 128 * b : 128 * (b + 1)].rearrange(
                "p (o j) -> p o j", o=1
            )
            i_b2, c_b2 = broadcast_tensor_aps(io3, crow3)
            nc.vector.tensor_tensor(cb3, i_b2, c_b2, mybir.AluOpType.is_equal)
            outp = o_psum.tile([128, 257], F32, tag="outp")
            for t in range(n_acc):
                for part in (ahi_tiles, alo_tiles):
                    nc.tensor.matmul(
                        outp[:, :],
                        cb[:, 128 * t : 128 * (t + 1)],
                        part[t][:, :],
                        start=(t == 0 and part is ahi_tiles),
                        stop=(t == n_acc - 1 and part is alo_tiles),
                        skip_group_check=True,
                    )
            eps = fin_pool.tile([128, 1], F32, tag="eps")
            nc.vector.tensor_scalar(
                eps[:], outp[:, D : D + 1], 1e-30, None, mybir.AluOpType.add,
            )
            rec = fin_pool.tile([128, 1], F32, tag="rec")
            nc.vector.reciprocal(rec[:], eps[:])
            ot = fin_pool.tile([128, D], F32, tag="ot")
            nc.vector.tensor_scalar(
                ot[:], outp[:, 0:D], rec[:, 0:1], None, mybir.AluOpType.mult,
            )
            nc.sync.dma_start(out[128 * b : 128 * (b + 1), :], ot[:])

    return nc


def kernel(x, batch, W1, b1, W2, b2):
    x = np.asarray(x, dtype=np.float32)
    batch = np.asarray(batch)
    W1 = np.asarray(W1, dtype=np.float32)
    b1 = np.asarray(b1, dtype=np.float32)
    W2 = np.asarray(W2, dtype=np.float32)
    b2 = np.asarray(b2, dtype=np.float32)
    n, d = x.shape
    assert d == D

    bounds = np.searchsorted(batch, np.arange(NUM_SEGMENTS + 1))
    core_starts = [int(bounds[SEGS_PER_CORE * m]) for m in range(N_CORES + 1)]
    rows_per_core = [core_starts[m + 1] - core_starts[m] for m in range(N_CORES)]
    n_groups = max(1, int(np.ceil(max(rows_per_core) / G_ROWS)))
    r_pad = n_groups * G_ROWS
    n_tiles = n_groups * TILES_PER_G

    # window width: 32 segs if every group's span fits unaligned, else 64
    # (32-aligned start).
    max_span = 0
    for m in range(N_CORES):
        rs, re = core_starts[m], core_starts[m + 1]
        seg_local = batch[rs:re] - SEGS_PER_CORE * m
        rows = re - rs
        for g in range(n_groups):
            lo = g * G_ROWS
            hi = min((g + 1) * G_ROWS, rows)
            if lo < rows:
                max_span = max(
                    max_span, int(seg_local[hi - 1]) - int(seg_local[lo])
                )
    w_seg = 32 if max_span < 32 else 64
    wins_per_acc = 128 // w_seg
    n_acc = -(-n_groups // wins_per_acc)
    n_segb = SEGS_PER_CORE // 128

    # shared constant inputs
    w1c = np.ascontiguousarray(W1.reshape(2, 128, H).astype(ml_dtypes.bfloat16))
    w2col = np.ascontiguousarray(W2.reshape(H, 1).astype(ml_dtypes.bfloat16))
    b1col = np.ascontiguousarray(b1.reshape(H, 1))
    iotaw = np.broadcast_to(np.arange(w_seg), (128, w_seg)).astype(ml_dtypes.bfloat16)
    iotas = np.broadcast_to(
        np.arange(n_segb * 128), (128, n_segb * 128)
    ).astype(np.float32).copy()
    b2_val = float(b2.reshape(-1)[0])

    in_maps = []
    for m in range(N_CORES):
        rs, re = core_starts[m], core_starts[m + 1]
        rows = re - rs
        xm = x[rs:re]
        x_flat = np.zeros((r_pad, D + 2), dtype=ml_dtypes.bfloat16)
        x_flat[:rows, :D] = xm.astype(ml_dtypes.bfloat16)
        x_flat[:rows, D] = ml_dtypes.bfloat16(1.0)
        # partition-major: x_nat[p, t, :] = x_flat[128t + p, :]
        x_nat = np.ascontiguousarray(
            x_flat.reshape(n_tiles, 128, D + 2).transpose(1, 0, 2)
        )
        xT = np.zeros((D, r_pad), dtype=ml_dtypes.bfloat16)
        xT[:, :rows] = xm.T.astype(ml_dtypes.bfloat16)

        seg_local = (batch[rs:re] - SEGS_PER_CORE * m).astype(np.int64)
        assert seg_local.min() >= 0 and seg_local.max() < SEGS_PER_CORE

        bl = np.full((128, n_tiles), PAD_BL, dtype=np.float32)
        # one-hot fp8 window mask: m8[p, t, s] = (bl[p, t] == s)
        crow_h = np.full((128, n_acc), PAD_SEG, dtype=np.float32)
        for g in range(n_groups):
            lo = g * G_ROWS
            hi = min((g + 1) * G_ROWS, rows)
            if lo >= rows:
                s0 = SEGS_PER_CORE + 128  # pad region, never selected
            else:
                if w_seg == 32:
                    s0 = int(seg_local[lo])
                else:
                    s0 = 32 * (int(seg_local[lo]) // 32)
                span = int(seg_local[hi - 1]) - s0
                assert span < w_seg, f"group seg span {span} >= {w_seg}"
                rr = np.arange(lo, hi)
                p = rr % 128
                c = (rr % G_ROWS) // 128
                bl[p, g * TILES_PER_G + c] = (seg_local[lo:hi] - s0).astype(np.float32)
            j = g % wins_per_acc
            crow_h[w_seg * j : w_seg * (j + 1), g // wins_per_acc] = (
                s0 + np.arange(w_seg)
            )

        in_maps.append(
            {
                "x_nat": x_nat,
                "xT": xT,
                "w1c": w1c,
                "w2col": w2col,
                "b1col": b1col,
                "iotaw": iotaw,
                "bl_all": bl,
                "crow": crow_h,
                "iotas": iotas,
                "zpad": np.zeros((128, 257), dtype=np.float32),
            }
        )

    nc = build_nc(n_groups, b2_val, w_seg)
    if not nc.is_finalized():
        nc.finalize()
    trace = os.environ.get("KERNEL_TRACE", "0") == "1"
    kw = {}
    if trace:
        kw = dict(trace=True, tmpdir=os.environ.get("KERNEL_TRACE_DIR") or None)
    res = run_bass_kernel_spmd(nc, in_maps, core_ids=list(range(N_CORES)), **kw)
    global LAST_EXEC_NS
    LAST_EXEC_NS = res.exec_time_ns
    if trace:
        print(
            f"exec_time_ns={res.exec_time_ns} mean={res.mean_exec_time_ns} "
            f"max_core={res.max_exec_time_core_id}",
            flush=True,
        )
    outs = res.results

    full = np.empty((NUM_SEGMENTS, D), dtype=np.float32)
    for m in range(N_CORES):
        full[SEGS_PER_CORE * m : SEGS_PER_CORE * (m + 1)] = outs[m]["out"]
    return full
